# revision 36
# baseline (speedup 1.0000x reference)
"""Trainium2 Bass kernel for nn_AttentionBlock (GroupNorm -> QKV -> MHA -> proj -> residual).

Full inputs in, full output out. Sharding: 8 cores = 4 batches x 2 head-pairs.
Each core computes GroupNorm(x_b), its 2 heads' q/k/v projections, attention,
and a partial output projection over its 2 heads. Host sums the two partials
per batch and adds bias + residual.

The attention path runs in fp8e4m3 with DoubleRow matmuls (2 contraction
subtiles per pass): QKV projections, scores (hd split 64x2 across
partitions/dim1), exp(score-1) weights, A@V, denominator (ones matmul), and
the output projection. The residual dominates the output 55:1, so fp8 on the
attention branch keeps total rel err ~1e-3 against the 2e-2 gate.

Self-contained: hardcodes shapes from the problem spec
(x: (4, 512, 64, 64) fp32, weights 512x512, 4 heads, 32 groups, eps 1e-5).
"""
import sys
import numpy as np

if '/opt/trn_rl_repo' not in sys.path:
    sys.path.insert(0, '/opt/trn_rl_repo')

N_CORES = 8
B, C, H, W = 4, 512, 64, 64
HW = H * W            # 4096
NH, NG, EPS = 4, 32, 1e-5
HD = C // NH          # 128 head dim
SCALE = 1.0 / float(np.sqrt(HD))
NKC = HW // 128       # 32 k-chunks of 128
NQG = HW // 512       # 8 q-groups of 512


# Measured on HW (rep-slope 128 vs 512): all-ACT exp = 540us/rep beats the
# DVE/Pool fast-exp2 split (598us) and the f32r baseline (577us); GPSIMD
# software ops are far slower on silicon than the cost model suggests.
USE_SPLIT = False     # fast-exp2 offload to DVE/Pool
USE_POOL_XN = False   # GroupNorm normalize on Pool for even chunks
# i16 fast-exp2 on DVE with U/D reading the bf16-bitcast view: also measured
# slower on HW (606us vs 565us same-session) — DVE offload never pays here.
USE_SPLIT16 = False


def _build_program(reps=1):
    import contextlib
    import concourse.bacc as bacc
    import concourse.bass as bass
    import concourse.tile as tile
    import concourse.mybir as mybir
    from concourse import masks

    f32, bf16 = mybir.dt.float32, mybir.dt.bfloat16
    fp8 = mybir.dt.float8e4
    i32 = mybir.dt.int32
    i16 = mybir.dt.int16
    AF = mybir.ActivationFunctionType
    OP = mybir.AluOpType
    DR = mybir.MatmulPerfMode.DoubleRow

    # Softmax weights are exp(s*SCALE + EXPB); the shift cancels in U/D and
    # keeps the max weight (~exp(6.93 + EXPB)) well under fp8e4m3's 240 max.
    EXPB = -2.5
    # fast-exp2 (Schraudolph): bits(y) = s*C1 + C2 read as f32 approximates
    # exp(s*SCALE + EXPB); bf16 view of the high bytes feeds the fp8 convert.
    LOG2E = float(np.log2(np.e))
    C1 = SCALE * LOG2E * (2.0 ** 23)
    C2 = (127.0 + EXPB * LOG2E) * (2.0 ** 23) - 366393.0
    # i16 variant: same trick at 2^7 scale; bitcast i16 -> bf16 IS the value
    C1_16 = SCALE * LOG2E * (2.0 ** 7)
    C2_16 = (127.0 + EXPB * LOG2E) * (2.0 ** 7) - 366393.0 / (2.0 ** 16)

    nc = bacc.Bacc("TRN2", target_bir_lowering=False, debug=False, num_devices=1)

    # fp8 inputs travel as uint8 and are bitcast on the DMA in.
    x_d = nc.dram_tensor("x", [C, HW], f32, kind="ExternalInput").ap()
    wq_d = nc.dram_tensor("wq8", [C, 2 * HD], mybir.dt.uint8, kind="ExternalInput").ap().bitcast(fp8)
    wk_d = nc.dram_tensor("wk8", [C, 2 * HD], mybir.dt.uint8, kind="ExternalInput").ap().bitcast(fp8)
    wv_d = nc.dram_tensor("wv8", [C, 2 * HD], mybir.dt.uint8, kind="ExternalInput").ap().bitcast(fp8)
    wp_d = nc.dram_tensor("wp8", [2 * HD, C], mybir.dt.uint8, kind="ExternalInput").ap().bitcast(fp8)
    gnw_d = nc.dram_tensor("gnw", [4, 128, 1], f32, kind="ExternalInput").ap()
    gnb_d = nc.dram_tensor("gnb", [4, 128, 1], f32, kind="ExternalInput").ap()
    bq_d = nc.dram_tensor("bq", [2, 128, 1], f32, kind="ExternalInput").ap()
    bk_d = nc.dram_tensor("bk", [2, 128, 1], f32, kind="ExternalInput").ap()
    bv_d = nc.dram_tensor("bv", [2, 128, 1], f32, kind="ExternalInput").ap()
    gt_d = nc.dram_tensor("gt", [C, NG], f32, kind="ExternalInput").ap()
    ex_d = nc.dram_tensor("ex", [NG, 4, 128], f32, kind="ExternalInput").ap()
    out_d = nc.dram_tensor("out_part", [C, HW], f32, kind="ExternalOutput").ap()

    with tile.TileContext(nc) as tc:
      with (tc.For_i(0, reps, 1) if reps > 1 else contextlib.nullcontext()):
        with tc.tile_pool(name="consts", bufs=1) as consts, \
             tc.tile_pool(name="persist", bufs=1) as persist:
            ident_bf = consts.tile([128, 128], bf16)
            masks.make_identity(nc, ident_bf)
            ones8 = consts.tile([128, 2, 128], fp8)
            nc.vector.memset(ones8, 1.0)
            ones16 = consts.tile([128, 128], bf16)
            nc.vector.memset(ones16, 1.0)
            eps_t = consts.tile([128, 1], f32)
            nc.vector.memset(eps_t, EPS)
            nbias = consts.tile([128, 1], f32)
            nc.vector.memset(nbias, EXPB)
            wp_t = consts.tile([128, 2, C], fp8)
            nc.sync.dma_start(out=wp_t, in_=wp_d.rearrange("(t p) c -> p t c", p=128))

            # Persistent per-head activations (all attention-side fp8)
            xn8 = persist.tile([128, 4, HW], fp8, name="xn8")
            q8 = [persist.tile([128, HW], fp8, tag=f"q{h}", name=f"q{h}") for h in range(2)]
            k8 = [persist.tile([128, HW], fp8, tag=f"k{h}", name=f"k{h}") for h in range(2)]
            v16 = [persist.tile([128, HW], bf16, tag=f"v{h}", name=f"v{h}") for h in range(2)]
            vT8 = [persist.tile([128, NKC, 128], fp8, tag=f"vT{h}", name=f"vT{h}") for h in range(2)]
            qdr = [persist.tile([64, 2, HW], fp8, tag=f"qdr{h}", name=f"qdr{h}") for h in range(2)]
            kdr = [persist.tile([64, 2, HW], fp8, tag=f"kdr{h}", name=f"kdr{h}") for h in range(2)]
            pt2 = [persist.tile([128, NKC, 512], fp8, tag=f"pt{h}", name=f"pt{h}") for h in range(2)]
            un8 = persist.tile([128, 2, 512], fp8, name="un8")

            # ---------------- Phase A: load x, GroupNorm, QKV, vT ----------------
            with tc.tile_pool(name="aw", bufs=1) as aw, \
                 tc.tile_pool(name="xp", bufs=2) as xp, \
                 tc.tile_pool(name="small", bufs=6) as small, \
                 tc.tile_pool(name="psA", bufs=3, space="PSUM") as psA, \
                 tc.tile_pool(name="psT", bufs=2, space="PSUM") as psT, \
                 tc.tile_pool(name="psG", bufs=1, space="PSUM") as psG:

                wq_t = aw.tile([128, 4, 2 * HD], fp8, tag="wq")
                wk_t = aw.tile([128, 4, 2 * HD], fp8, tag="wk")
                wv_t = aw.tile([128, 4, 2 * HD], fp8, tag="wv")
                nc.sync.dma_start(out=wq_t, in_=wq_d.rearrange("(t p) c -> p t c", p=128))
                nc.sync.dma_start(out=wk_t, in_=wk_d.rearrange("(t p) c -> p t c", p=128))
                nc.sync.dma_start(out=wv_t, in_=wv_d.rearrange("(t p) c -> p t c", p=128))
                gnw_t = aw.tile([128, 4, 1], f32, tag="gnw")
                gnb_t = aw.tile([128, 4, 1], f32, tag="gnb")
                nc.sync.dma_start(out=gnw_t, in_=gnw_d.rearrange("t p one -> p t one"))
                nc.sync.dma_start(out=gnb_t, in_=gnb_d.rearrange("t p one -> p t one"))
                bq_t = aw.tile([128, 2, 1], f32, tag="bq")
                bk_t = aw.tile([128, 2, 1], f32, tag="bk")
                bv_t = aw.tile([128, 2, 1], f32, tag="bv")
                nc.sync.dma_start(out=bq_t, in_=bq_d.rearrange("h p one -> p h one"))
                nc.sync.dma_start(out=bk_t, in_=bk_d.rearrange("h p one -> p h one"))
                nc.sync.dma_start(out=bv_t, in_=bv_d.rearrange("h p one -> p h one"))
                gt_t = aw.tile([128, 4, NG], f32, tag="gt")
                nc.sync.dma_start(out=gt_t, in_=gt_d.rearrange("(t p) g -> p t g", p=128))
                ex_t = aw.tile([NG, 4, 128], f32, tag="ex")
                nc.sync.dma_start(out=ex_t, in_=ex_d)

                # per-chunk GroupNorm: stats, group reduce, affine, fp8 normalize
                for t in range(4):
                    xt = xp.tile([128, HW], f32, tag="x")
                    # two DMA queues in parallel per chunk
                    nc.sync.dma_start(out=xt[:, 0:HW // 2],
                                      in_=x_d[t * 128:(t + 1) * 128, 0:HW // 2])
                    nc.gpsimd.dma_start(out=xt[:, HW // 2:],
                                        in_=x_d[t * 128:(t + 1) * 128, HW // 2:])
                    st = small.tile([128, 8, 6], f32, tag="bnst")
                    for chk in range(8):
                        nc.vector.bn_stats(out=st[:, chk, :],
                                           in_=xt[:, chk * 512:(chk + 1) * 512])
                    mv = small.tile([128, 2], f32, tag="mv")
                    nc.vector.bn_aggr(out=mv, in_=st)
                    # me = [mean, E[x^2]] = [mean, var + mean^2]
                    me = small.tile([128, 2], f32, tag="me")
                    m2 = small.tile([128, 1], f32, tag="m2")
                    nc.vector.tensor_copy(me[:, 0:1], mv[:, 0:1])
                    nc.vector.tensor_mul(m2, mv[:, 0:1], mv[:, 0:1])
                    nc.vector.tensor_add(me[:, 1:2], mv[:, 1:2], m2)

                    gps = psG.tile([NG, 2], f32, tag="gps")
                    nc.tensor.matmul(gps, gt_t[:, t, :], me, start=True, stop=True)
                    gsb = small.tile([NG, 2], f32, tag="gsb")
                    nc.vector.tensor_copy(gsb, gps)
                    m2g = small.tile([NG, 1], f32, tag="m2g")
                    var_g = small.tile([NG, 1], f32, tag="varg")
                    nc.vector.tensor_mul(m2g, gsb[:, 0:1], gsb[:, 0:1])
                    nc.vector.tensor_sub(var_g, gsb[:, 1:2], m2g)
                    sd_g = small.tile([NG, 1], f32, tag="sdg")
                    nc.scalar.activation(out=sd_g, in_=var_g, func=AF.Sqrt,
                                         bias=eps_t[0:NG, :], scale=1.0)
                    rstd_g = small.tile([NG, 1], f32, tag="rstdg")
                    nc.vector.reciprocal(rstd_g, sd_g)
                    grp = small.tile([NG, 2], f32, tag="grp")
                    nc.vector.tensor_copy(grp[:, 0:1], gsb[:, 0:1])
                    nc.vector.tensor_copy(grp[:, 1:2], rstd_g)

                    bcp = psG.tile([128, 2], f32, tag="bcp")
                    nc.tensor.matmul(bcp, ex_t[:, t, :], grp, start=True, stop=True)
                    bc = small.tile([128, 2], f32, tag="bc")
                    nc.vector.tensor_copy(bc, bcp)
                    A_t = small.tile([128, 1], f32, tag="At")
                    tmp = small.tile([128, 1], f32, tag="tmp")
                    B_t = small.tile([128, 1], f32, tag="Bt")
                    nc.vector.tensor_mul(A_t, bc[:, 1:2], gnw_t[:, t, :])
                    nc.vector.tensor_mul(tmp, bc[:, 0:1], A_t)
                    nc.vector.tensor_sub(B_t, gnb_t[:, t, :], tmp)
                    # split the big normalize between DVE and ACT (idle in A)
                    if t % 2:
                        nc.vector.tensor_scalar(out=xn8[:, t, :], in0=xt,
                                                scalar1=A_t, scalar2=B_t,
                                                op0=OP.mult, op1=OP.add)
                    elif USE_POOL_XN:
                        nc.gpsimd.tensor_scalar(out=xn8[:, t, :], in0=xt,
                                                scalar1=A_t, scalar2=B_t,
                                                op0=OP.mult, op1=OP.add)
                    else:
                        nc.scalar.activation(out=xn8[:, t, :], in_=xt,
                                             func=AF.Identity,
                                             bias=B_t, scale=A_t)

                # QKV projections (fp8 DoubleRow over the 4 c-chunks).
                # k,q (+ their DR-relayout DMAs) go first so attention can
                # start while v projections / transposes continue.
                def proj(h, wname, wt, bt):
                    osb = {"k": k8, "q": q8, "v": v16}[wname][h]
                    for s in range(8):
                        pj = psA.tile([128, 512], f32, tag="pj")
                        for i in range(2):
                            nc.tensor.matmul(
                                pj,
                                wt[:, 2 * i:2 * i + 2, h * HD:(h + 1) * HD],
                                xn8[:, 2 * i:2 * i + 2, s * 512:(s + 1) * 512],
                                start=(i == 0), stop=(i == 1), perf_mode=DR)
                        # bias-add + dtype convert on ACT (idle in phase A)
                        nc.scalar.activation(
                            out=osb[:, s * 512:(s + 1) * 512], in_=pj,
                            func=AF.Identity, bias=bt[:, h, :], scale=1.0)
                        if wname == "v":
                            for kc in range(4 * s, 4 * s + 4):
                                pv = psT.tile([128, 128], bf16, tag="pvt")
                                nc.tensor.transpose(
                                    pv, v16[h][:, kc * 128:(kc + 1) * 128],
                                    ident_bf)
                                # DVE only: keep ACT exp-only once phase B runs
                                nc.vector.tensor_copy(vT8[h][:, kc, :], pv)
                    if wname == "k":
                        nc.sync.dma_start(out=kdr[h][:, 0, :], in_=k8[h][0:64, :])
                        nc.gpsimd.dma_start(out=kdr[h][:, 1, :], in_=k8[h][64:128, :])
                    elif wname == "q":
                        nc.sync.dma_start(out=qdr[h][:, 0, :], in_=q8[h][0:64, :])
                        nc.gpsimd.dma_start(out=qdr[h][:, 1, :], in_=q8[h][64:128, :])

                for h in range(2):
                    proj(h, "k", wk_t, bk_t)
                    proj(h, "q", wq_t, bq_t)
                for h in range(2):
                    proj(h, "v", wv_t, bv_t)

            # ---------------- Phase B: attention + output projection ----------------
            with tc.tile_pool(name="psS", bufs=2, space="PSUM") as psS, \
                 tc.tile_pool(name="psUD", bufs=2, space="PSUM") as psUD, \
                 tc.tile_pool(name="ob", bufs=2) as ob, \
                 tc.tile_pool(name="ybp", bufs=(8 if USE_SPLIT16 else 3)) as ybp, \
                 tc.tile_pool(name="mb", bufs=2) as mb:

                # per score-tile: 3 kpos chunks (last tile 2); 11 tiles cover 32
                TL = [3] * 10 + [2]
                # softmax-weight producer per tile: 'A' = ACT exp -> fp8 pt2;
                # 'D' = DVE i16 fast-exp2, U/D read the bf16-bitcast directly
                if USE_SPLIT16:
                    ASSIGN = ['A', 'A', 'D', 'A', 'A', 'D', 'A', 'A', 'D', 'A', 'A']
                elif USE_SPLIT:
                    ASSIGN = ['A', 'D', 'A', 'D', 'A', 'D', 'A', 'D', 'A', 'D', 'A']
                else:
                    ASSIGN = ['A'] * 11
                # consumer op plan: DR pairs over contiguous ACT chunks, fp8
                # singles for leftovers, bf16 singles for DVE-i16 chunks
                plan = []
                run = []

                def _flush():
                    k = 0
                    while k + 1 < len(run):
                        plan.append(('dr', run[k]))
                        k += 2
                    if k < len(run):
                        plan.append(('s8', run[k]))
                    del run[:]

                for t in range(11):
                    if ASSIGN[t] == 'A':
                        run.extend(range(3 * t, 3 * t + TL[t]))
                    else:
                        _flush()
                        for j in range(TL[t]):
                            plan.append(('s16', t, j))
                _flush()
                allops = [('U',) + op for op in plan] + [('D',) + op for op in plan]
                segs = [allops[(len(allops) * s) // 16:(len(allops) * (s + 1)) // 16]
                        for s in range(16)]
                ybs = {0: {}, 1: {}}

                for i in range(17):
                    prod = i < 16
                    cons = i >= 1
                    if prod:
                        h, g = i % 2, i // 2
                        qs = qdr[h][:, :, g * 512:(g + 1) * 512]
                    if cons:
                        h2, g2 = (i - 1) % 2, (i - 1) // 2
                        U = psUD.tile([128, 512], f32, tag="ud")
                        D = psUD.tile([128, 512], f32, tag="ud")
                        nops = {'U': 0, 'D': 0}

                    def _emit(op):
                        which = op[0]
                        acc = U if which == 'U' else D
                        st = nops[which] == 0
                        sp = nops[which] == len(plan) - 1
                        nops[which] += 1
                        if op[1] == 'dr':
                            c0 = op[2]
                            lhs = (vT8[h2][:, c0:c0 + 2, :] if which == 'U'
                                   else ones8)
                            nc.tensor.matmul(acc, lhs,
                                             pt2[h2][:, c0:c0 + 2, :],
                                             start=st, stop=sp, perf_mode=DR)
                        elif op[1] == 's8':
                            c = op[2]
                            lhs = (vT8[h2][:, c, :] if which == 'U'
                                   else ones8[:, 0, :])
                            nc.tensor.matmul(acc, lhs, pt2[h2][:, c, :],
                                             start=st, stop=sp)
                        else:  # s16
                            t16, j = op[2], op[3]
                            yb16 = ybs[h2][t16]
                            lhs = (vT8[h2][:, 3 * t16 + j, :] if which == 'U'
                                   else ones16)
                            nc.tensor.matmul(acc, lhs,
                                             yb16.bitcast(bf16)[:, j, :],
                                             start=st, stop=sp)

                    # interleave scores(i) tiles with U/D(i-1) matmuls
                    for t in range(16):
                        if prod and t < 11:
                            ps = psS.tile([128, 3, 512], f32, tag="s")
                            kc0 = 3 * t
                            ln = TL[t]
                            for j in range(ln):
                                kc = kc0 + j
                                nc.tensor.matmul(
                                    ps[:, j, :],
                                    kdr[h][:, :, kc * 128:(kc + 1) * 128],
                                    qs, start=True, stop=True, perf_mode=DR)
                            if ASSIGN[t] == 'A':
                                nc.scalar.activation(
                                    out=pt2[h][:, kc0:kc0 + ln, :],
                                    in_=ps[:, 0:ln, :], func=AF.Exp,
                                    bias=nbias, scale=SCALE)
                            elif USE_SPLIT16:
                                yb = ybp.tile([128, 3, 512], i16, tag="yb")
                                nc.vector.tensor_scalar(
                                    out=yb[:, 0:ln, :], in0=ps[:, 0:ln, :],
                                    scalar1=C1_16, scalar2=C2_16,
                                    op0=OP.mult, op1=OP.add)
                                ybs[h][t] = yb
                            else:
                                yb = ybp.tile([128, 3, 512], i32, tag="yb")
                                nc.vector.tensor_scalar(
                                    out=yb[:, 0:ln, :], in0=ps[:, 0:ln, :],
                                    scalar1=C1, scalar2=C2,
                                    op0=OP.mult, op1=OP.add)
                                nc.gpsimd.tensor_copy(
                                    pt2[h][:, kc0:kc0 + ln, :],
                                    yb.bitcast(bf16)[:, 0:ln, 1::2])
                        if cons:
                            for op in segs[t]:
                                _emit(op)
                    if cons:
                        recD = mb.tile([128, 512], f32, tag="recd")
                        nc.vector.reciprocal(recD, D)
                        nc.vector.tensor_mul(un8[:, h2, :], U, recD)
                        if h2 == 1:
                            ost = ob.tile([128, 4, 512], f32, tag="ost")
                            for m in range(4):
                                pp = psUD.tile([128, 512], f32, tag="ud")
                                nc.tensor.matmul(
                                    pp, wp_t[:, :, m * 128:(m + 1) * 128],
                                    un8, start=True, stop=True, perf_mode=DR)
                                # keep phase-B ACT exp-only; DVE has slack
                                nc.vector.tensor_copy(ost[:, m, :], pp)
                            nc.sync.dma_start(
                                out=out_d[:, g2 * 512:(g2 + 1) * 512]
                                    .rearrange("(m p) c -> p m c", p=128),
                                in_=ost)

    nc.compile()
    return nc


def _get_program():
    import concourse  # noqa: F401  (ensure import works before caching)
    global _PROGRAM
    try:
        return _PROGRAM
    except NameError:
        _PROGRAM = _build_program()
        return _PROGRAM


def _host_prep(inputs):
    import ml_dtypes
    E4 = ml_dtypes.float8_e4m3

    def q8(a):
        return np.ascontiguousarray(a.astype(E4)).view(np.uint8)

    x = np.ascontiguousarray(np.asarray(inputs["x"], dtype=np.float32))
    wq = np.asarray(inputs["wq"], dtype=np.float32)
    wk = np.asarray(inputs["wk"], dtype=np.float32)
    wv = np.asarray(inputs["wv"], dtype=np.float32)
    wp = np.asarray(inputs["wp"], dtype=np.float32)
    gnw = np.asarray(inputs["gn_w"], dtype=np.float32).reshape(4, 128, 1)
    gnb = np.asarray(inputs["gn_b"], dtype=np.float32).reshape(4, 128, 1)
    bq = np.asarray(inputs["bq"], dtype=np.float32)
    bk = np.asarray(inputs["bk"], dtype=np.float32)
    bv = np.asarray(inputs["bv"], dtype=np.float32)
    gt = np.zeros((C, NG), dtype=np.float32)
    gt[np.arange(C), np.arange(C) // (C // NG)] = 1.0 / (C // NG)
    ex = np.zeros((NG, 4, 128), dtype=np.float32)
    for t in range(4):
        cl = np.arange(128)
        ex[8 * t + cl // 16, t, cl] = 1.0

    in_maps = []
    for core in range(N_CORES):
        b, p = core // 2, core % 2
        ch0 = 2 * HD * p
        in_maps.append({
            "x": np.ascontiguousarray(x[b].reshape(C, HW)),
            "wq8": q8(wq[ch0:ch0 + 2 * HD, :].T),
            "wk8": q8(wk[ch0:ch0 + 2 * HD, :].T),
            "wv8": q8(wv[ch0:ch0 + 2 * HD, :].T),
            "wp8": q8(wp[:, ch0:ch0 + 2 * HD].T),
            "gnw": gnw, "gnb": gnb,
            "bq": np.ascontiguousarray(bq[ch0:ch0 + 2 * HD].reshape(2, 128, 1)),
            "bk": np.ascontiguousarray(bk[ch0:ch0 + 2 * HD].reshape(2, 128, 1)),
            "bv": np.ascontiguousarray(bv[ch0:ch0 + 2 * HD].reshape(2, 128, 1)),
            "gt": gt, "ex": ex,
        })
    return x, in_maps


def kernel(**inputs):
    from concourse.bass_utils import run_bass_kernel_spmd
    x, in_maps = _host_prep(inputs)
    bp = np.asarray(inputs["bp"], dtype=np.float32)
    nc = _get_program()
    res = run_bass_kernel_spmd(nc, in_maps, core_ids=list(range(N_CORES)))
    parts = [res.results[c]["out_part"] for c in range(N_CORES)]
    out = np.empty((B, C, HW), dtype=np.float32)
    for b in range(B):
        out[b] = (x[b].reshape(C, HW) + bp[:, None]
                  + parts[2 * b] + parts[2 * b + 1])
    return out.reshape(B, C, H, W)
